# revision 1
# baseline (speedup 1.0000x reference)
"""Sliding-window GQA attention (T=4096, DIM=2048, H=16, KVH=4, D=128, W=1024)
as an 8-core SPMD Trainium2 Bass/Tile kernel.

Sharding: sequence-parallel. Core c owns queries [512c, 512c+512) and
recomputes K/V for its sliding window (1536 kv slots, zero-padded before
position 0). No collectives.

Dataflow (everything transposed so softmax needs no cross-partition max):
  Q^T[h] [d=128, q=512]   = RoPE(Wq_h^T x_q^T)        (per head)
  K^T[kvh] [128, 1536]    = RoPE(Wk_kvh^T x_kv^T)
  V[m] [t=128, 512=kvh*d] = x_kv[tile]^T^T ... natural layout per t-tile
  S^T [t-tile, q-span]    = K-tile(stationary) @ Q^T   (PSUM)
  P^T = exp(scale*S^T + kbias[t])   (ACT, fp32r out; kbias kills padded t)
  P^T *= triangle masks on boundary blocks (DVE)
  Y^T[h] += V-tile @ P^T ; den[h] += ones @ P^T        (PSUM accumulate)
  Y^T[h] = Y^T * (1/den)                               (softmax normalize)
  O^T[e-tile] += Wo-chunk(stationary) @ Y^T[h]         -> DRAM [2048, 512]

Host side: transposes, fp32r rounding (11-bit mantissa), RoPE tables with
sign-folded sin, masks, gather/unTranspose of per-core outputs.
"""

import math
import os
import sys

import numpy as np


def _ensure_paths():
    for p in (
        "/root/.axon_site",
        "/root/.axon_site/_ro/trn_rl_repo",
        "/root/.axon_site/_ro/pypackages",
        "/opt/trn_rl_repo",
        "/opt/pypackages",
    ):
        if os.path.isdir(p) and p not in sys.path:
            sys.path.append(p)


try:
    import concourse.bass as bass  # noqa: F401
except ImportError:
    _ensure_paths()

import concourse.bass as bass
import concourse.mybir as mybir
import concourse.tile as tile
from concourse import bacc
from concourse.bass_utils import run_bass_kernel_spmd

# ---------------------------------------------------------------- constants
N_CORES = 8
T = 4096
DIM = 2048
H = 16
KVH = 4
D = 128
WIN = 1024
ROPE_BASE = 10000.0

TQ = T // N_CORES          # 512 queries per core
TKV = TQ + WIN             # 1536 kv slots per core
NMT = TKV // 128           # 12 kv tiles of 128
NCC = DIM // 128           # 16 contraction chunks
SCALE = 1.0 / math.sqrt(D)
GQ = H // KVH              # 4 q heads per kv head

F32 = mybir.dt.float32
F32R = mybir.dt.float32r
BF16 = mybir.dt.bfloat16

# per kv-tile m: (qlo, qhi) span of local queries it can interact with
SPANS = {
    0: (0, 256), 1: (0, 256), 2: (0, 384), 3: (0, 512),
    4: (0, 512), 5: (0, 512), 6: (0, 512), 7: (0, 512),
    8: (0, 512), 9: (128, 512), 10: (256, 512), 11: (256, 512),
}
# per kv-tile m: (mask_name, local_lo, local_hi) or None
# per kv-tile m: (mask_name, lo, hi, zero_lo, zero_hi) in absolute q coords
MASKS = {
    0: ("maskB", 0, 128, 128, 256), 1: ("maskB", 128, 256, None, None),
    2: ("maskB", 256, 384, None, None), 3: ("maskB", 384, 512, None, None),
    4: None, 5: None, 6: None, 7: None,
    8: ("maskA", 0, 128, None, None), 9: ("maskA", 128, 256, None, None),
    10: ("maskA", 256, 384, None, None), 11: ("maskA", 384, 512, 256, 384),
}
# PSUM accumulation order: m=4 first (full-width span -> start=True clears
# the whole Y/den bank), m=11 last (stop=True).
M_ORDER = [4, 5, 6, 7, 0, 1, 2, 3, 8, 9, 10, 11]


def round_f32r(x):
    """fp32 -> fp32r: round-to-nearest-even to 11 mantissa bits."""
    b = np.ascontiguousarray(x, dtype=np.float32).view(np.uint32)
    b = (b + np.uint32(0x7FF) + ((b >> np.uint32(12)) & np.uint32(1))) & np.uint32(
        0xFFFFF000
    )
    return b.view(np.float32)


# ---------------------------------------------------------------- device code
_NC_CACHE = None


def _build():
    global _NC_CACHE
    if _NC_CACHE is not None:
        return _NC_CACHE

    nc = bacc.Bacc("TRN2", target_bir_lowering=False, debug=False,
                   num_devices=N_CORES)

    # DRAM I/O (per-core contents supplied via in_maps)
    xqT = nc.dram_tensor("xqT", [DIM, TQ], F32R, kind="ExternalInput").ap()
    xkvT = nc.dram_tensor("xkvT", [3 * DIM, 512], F32R, kind="ExternalInput").ap()
    wq = nc.dram_tensor("wq", [8 * DIM, 256], F32R, kind="ExternalInput").ap()
    wk = nc.dram_tensor("wk", [DIM, KVH * D], F32R, kind="ExternalInput").ap()
    wv = nc.dram_tensor("wv", [DIM, KVH * D], F32R, kind="ExternalInput").ap()
    wo = nc.dram_tensor("wo", [8 * DIM, 256], F32R, kind="ExternalInput").ap()
    cosq = nc.dram_tensor("cosq", [D, TQ], F32, kind="ExternalInput").ap()
    sinq = nc.dram_tensor("sinq", [D, TQ], F32, kind="ExternalInput").ap()
    cosk = nc.dram_tensor("cosk", [3 * D, 512], F32, kind="ExternalInput").ap()
    sink = nc.dram_tensor("sink", [3 * D, 512], F32, kind="ExternalInput").ap()
    kbias = nc.dram_tensor("kbias", [128, NMT], F32, kind="ExternalInput").ap()
    maskB = nc.dram_tensor("maskB", [128, 128], F32, kind="ExternalInput").ap()
    maskA = nc.dram_tensor("maskA", [128, 128], F32, kind="ExternalInput").ap()
    rotp = nc.dram_tensor("rotp", [128, 128], F32R, kind="ExternalInput").ap()
    ones = nc.dram_tensor("ones", [128, 128], F32R, kind="ExternalInput").ap()
    outT = nc.dram_tensor("outT", [DIM, TQ], F32, kind="ExternalOutput").ap()

    mask_dram = {"maskB": maskB, "maskA": maskA}

    with tile.TileContext(nc) as tc:
        _emit(nc, tc, xqT, xkvT, wq, wk, wv, wo, cosq, sinq, cosk, sink,
              kbias, mask_dram, rotp, ones, outT)

    nc.compile()
    _NC_CACHE = nc
    return nc


def _emit(nc, tc, xqT, xkvT, wq, wk, wv, wo, cosq, sinq, cosk, sink,
          kbias, mask_dram, rotp, ones, outT):
    from contextlib import ExitStack

    ctx = ExitStack()
    with ctx:
        # pools
        consts = ctx.enter_context(tc.tile_pool(name="consts", bufs=1))
        xbuf = ctx.enter_context(tc.tile_pool(name="xbuf", bufs=18))
        wqp = ctx.enter_context(tc.tile_pool(name="wqp", bufs=3))
        wres = ctx.enter_context(tc.tile_pool(name="wres", bufs=NCC))
        wvp = ctx.enter_context(tc.tile_pool(name="wvp", bufs=6))
        wop = ctx.enter_context(tc.tile_pool(name="wop", bufs=8))
        qtp = ctx.enter_context(tc.tile_pool(name="qtp", bufs=4))
        ktp = ctx.enter_context(tc.tile_pool(name="ktp", bufs=KVH))
        vp = ctx.enter_context(tc.tile_pool(name="vp", bufs=NMT))
        ytp = ctx.enter_context(tc.tile_pool(name="ytp", bufs=H))
        pp = ctx.enter_context(tc.tile_pool(name="pp", bufs=2))
        tmp = ctx.enter_context(tc.tile_pool(name="tmp", bufs=2))
        t12 = ctx.enter_context(tc.tile_pool(name="t12", bufs=3))
        fin = ctx.enter_context(tc.tile_pool(name="fin", bufs=2))
        ps_a = ctx.enter_context(tc.tile_pool(name="ps_a", bufs=2, space="PSUM"))
        ps_b = ctx.enter_context(tc.tile_pool(name="ps_b", bufs=2, space="PSUM"))
        ps_s = ctx.enter_context(tc.tile_pool(name="ps_s", bufs=2, space="PSUM"))
        ps_y = ctx.enter_context(tc.tile_pool(name="ps_y", bufs=2, space="PSUM"))

        Exp = mybir.ActivationFunctionType.Exp

        # ---- constants into SBUF
        def cload(ap, shape, dtype, tag):
            t = consts.tile(shape, dtype, tag=tag)
            nc.sync.dma_start(t[:], ap[:])
            return t

        rotp_sb = cload(rotp, [128, 128], F32R, "rotp")
        ones_sb = cload(ones, [128, 128], F32R, "ones")
        kbias_sb = cload(kbias, [128, NMT], F32, "kbias")
        cosq_sb = cload(cosq, [D, TQ], F32, "cosq")
        sinq_sb = cload(sinq, [D, TQ], F32, "sinq")
        mask_sb = {
            name: cload(mask_dram[name], [128, 128], F32, name)
            for name in ("maskB", "maskA")
        }

        def rope(src_ps, sin_sl, cos_sl, dst_ap, width):
            """dst = src*cos + rot_half(src)*sin  (dst fp32r)."""
            s_sb = tmp.tile([128, 512], F32R, tag="ropesb")
            nc.vector.tensor_copy(s_sb[:, :width], src_ps[:, :width])
            r_ps = ps_b.tile([128, 512], F32, tag="ps_b")
            nc.tensor.matmul(r_ps[:, :width], rotp_sb[:], s_sb[:, :width],
                             start=True, stop=True)
            t1 = t12.tile([128, 512], F32, tag="t12")
            nc.vector.tensor_mul(t1[:, :width], r_ps[:, :width], sin_sl)
            t2 = t12.tile([128, 512], F32, tag="t12")
            nc.vector.tensor_mul(t2[:, :width], src_ps[:, :width], cos_sl)
            nc.vector.tensor_add(dst_ap, t1[:, :width], t2[:, :width])

        # ---- phase A: K^T (RoPE'd) and V over 3 spans of 512 kv slots
        kt_sb = [ktp.tile([128, TKV], F32R, tag="kt", name=f"kt{g}")
                 for g in range(KVH)]
        v_sb = [vp.tile([128, 512], F32R, tag="v", name=f"v{m}")
                for m in range(NMT)]
        wk_res = []
        for c in range(NCC):
            wkt = wres.tile([128, 512], F32R, tag="wres", name=f"wkres{c}")
            nc.gpsimd.dma_start(wkt[:], wk[c * 128:(c + 1) * 128, :])
            wk_res.append(wkt)

        for s in range(3):
            xs = []
            for c in range(NCC):
                xt = xbuf.tile([128, 512], F32R, tag="xb")
                nc.sync.dma_start(
                    xt[:], xkvT[s * DIM + c * 128:s * DIM + (c + 1) * 128, :])
                xs.append(xt)
            cosk_s = xbuf.tile([128, 512], F32, tag="xb")
            nc.sync.dma_start(cosk_s[:], cosk[s * 128:(s + 1) * 128, :])
            sink_s = xbuf.tile([128, 512], F32, tag="xb")
            nc.sync.dma_start(sink_s[:], sink[s * 128:(s + 1) * 128, :])

            # K^T projection: c-outer across 4 psum banks (wk slab DMAs)
            kps = [ps_s.tile([128, 512], F32, tag="ps_s", name=f"kps{s}_0"),
                   ps_s.tile([128, 512], F32, tag="ps_s", name=f"kps{s}_1"),
                   ps_y.tile([128, 512], F32, tag="ps_y", name=f"kps{s}_2"),
                   ps_y.tile([128, 512], F32, tag="ps_y", name=f"kps{s}_3")]
            for c in range(NCC):
                for g in range(KVH):
                    nc.tensor.matmul(kps[g][:],
                                     wk_res[c][:, g * 128:(g + 1) * 128],
                                     xs[c][:],
                                     start=(c == 0), stop=(c == NCC - 1))
            for g in range(KVH):
                rope(kps[g], sink_s[:], cosk_s[:],
                     kt_sb[g][:, s * 512:(s + 1) * 512], 512)

            # V projection (natural layout): c-outer across 4 psum banks
            vps = [ps_a.tile([128, 512], F32, tag="ps_a", name=f"vps{s}_0"),
                   ps_a.tile([128, 512], F32, tag="ps_a", name=f"vps{s}_1"),
                   ps_b.tile([128, 512], F32, tag="ps_b", name=f"vps{s}_2"),
                   ps_b.tile([128, 512], F32, tag="ps_b", name=f"vps{s}_3")]
            for c in range(NCC):
                wvt = wvp.tile([128, 512], F32R, tag="wv")
                nc.sync.dma_start(wvt[:], wv[c * 128:(c + 1) * 128, :])
                for tt in range(4):
                    nc.tensor.matmul(
                        vps[tt][:],
                        xs[c][:, tt * 128:(tt + 1) * 128],
                        wvt[:],
                        start=(c == 0), stop=(c == NCC - 1))
            for tt in range(4):
                nc.vector.tensor_copy(v_sb[4 * s + tt][:], vps[tt][:])

        # ---- phases B+C interleaved per head
        xq_sb = []
        for c in range(NCC):
            xt = xbuf.tile([128, 512], F32R, tag="xb")
            nc.sync.dma_start(xt[:], xqT[c * 128:(c + 1) * 128, :])
            xq_sb.append(xt)

        yt_sb = [ytp.tile([128, TQ], F32R, tag="yt", name=f"yt{h}")
                 for h in range(H)]

        qts = {}

        def emit_pair_proj(p_):
            h0 = 2 * p_
            qpair = [ps_a.tile([128, 512], F32, tag="ps_a",
                               name=f"qps{h0}_{j}") for j in range(2)]
            for c in range(NCC):
                wqt = wqp.tile([128, 256], F32R, tag="wq",
                               name=f"wqt{h0}_{c}")
                nc.gpsimd.dma_start(
                    wqt[:],
                    wq[p_ * DIM + c * 128:p_ * DIM + (c + 1) * 128, :])
                for j in range(2):
                    nc.tensor.matmul(qpair[j][:],
                                     wqt[:, j * 128:(j + 1) * 128],
                                     xq_sb[c][:],
                                     start=(c == 0), stop=(c == NCC - 1))
            for j in range(2):
                qtj = qtp.tile([128, TQ], F32R, tag="qt", name=f"qt{h0}_{j}")
                rope(qpair[j], sinq_sb[:], cosq_sb[:], qtj[:], TQ)
                qts[h0 + j] = qtj

        def emit_attn(h):
            g = h // GQ
            qt = qts[h]
            yps = ps_y.tile([128, TQ], F32, tag="ps_y", name=f"yps{h}")
            dps = ps_b.tile([128, TQ], F32, tag="ps_b", name=f"dps{h}")
            for mi, m in enumerate(M_ORDER):
                qlo, qhi = SPANS[m]
                w = qhi - qlo
                sps = ps_s.tile([128, 512], F32, tag="ps_s", name=f"sps{h}_{m}")
                nc.tensor.matmul(sps[:, :w],
                                 kt_sb[g][:, m * 128:(m + 1) * 128],
                                 qt[:, qlo:qhi], start=True, stop=True)
                p = pp.tile([128, 512], F32R, tag="p", name=f"p{h}_{m}")
                nc.scalar.activation(p[:, :w], sps[:, :w], Exp,
                                     bias=kbias_sb[:, m:m + 1], scale=SCALE)
                mk = MASKS[m]
                if mk is not None:
                    name, lo, hi, zlo, zhi = mk
                    nc.vector.tensor_mul(p[:, lo - qlo:hi - qlo],
                                         p[:, lo - qlo:hi - qlo],
                                         mask_sb[name][:])
                    if zlo is not None:
                        nc.vector.tensor_scalar_mul(
                            p[:, zlo - qlo:zhi - qlo],
                            p[:, zlo - qlo:zhi - qlo], 0.0)
                first = mi == 0
                last = mi == len(M_ORDER) - 1
                nc.tensor.matmul(yps[:, qlo:qhi],
                                 v_sb[m][:, g * 128:(g + 1) * 128],
                                 p[:, :w], start=first, stop=last)
                nc.tensor.matmul(dps[:, qlo:qhi], ones_sb[:], p[:, :w],
                                 start=first, stop=last)

            rcp = fin.tile([128, TQ], F32, tag="rcp", name=f"rcp{h}")
            nc.vector.reciprocal(rcp[:], dps[:])
            nc.vector.tensor_mul(yt_sb[h][:], yps[:], rcp[:])

        # one-pair lookahead: emit projections a pair ahead of attention
        emit_pair_proj(0)
        for p_ in range(H // 2):
            if p_ + 1 < H // 2:
                emit_pair_proj(p_ + 1)
            emit_attn(2 * p_)
            emit_attn(2 * p_ + 1)

        # ---- phase D: O^T projection in e-tile pairs
        for n0 in range(0, NCC, 2):
            opair = [ps_a.tile([128, 512], F32, tag="ps_a",
                               name=f"ops{n0}_{j}") for j in range(2)]
            for h in range(H):
                wot = wop.tile([128, 256], F32R, tag="wo")
                np_ = n0 // 2
                nc.sync.dma_start(
                    wot[:],
                    wo[np_ * DIM + h * 128:np_ * DIM + (h + 1) * 128, :])
                for j in range(2):
                    nc.tensor.matmul(opair[j][:],
                                     wot[:, j * 128:(j + 1) * 128],
                                     yt_sb[h][:],
                                     start=(h == 0), stop=(h == H - 1))
            for j in range(2):
                osb = fin.tile([128, TQ], F32, tag="osb")
                nc.vector.tensor_copy(osb[:], opair[j][:])
                nc.sync.dma_start(outT[(n0 + j) * 128:(n0 + j + 1) * 128, :],
                                  osb[:])


# ---------------------------------------------------------------- host side
def _host_inputs(x, Wq, Wk, Wv, Wo):
    x = np.asarray(x, dtype=np.float32).reshape(T, DIM)

    inv_freq = 1.0 / (ROPE_BASE ** (np.arange(0, D, 2, dtype=np.float64) / D))
    dfreq = np.concatenate([inv_freq, inv_freq])  # [128] per-dim freq

    wq_r = round_f32r(
        np.asarray(Wq).reshape(DIM, 8, 256).transpose(1, 0, 2).reshape(8 * DIM, 256))
    wk_r = round_f32r(Wk)
    wv_r = round_f32r(Wv)
    wo_r = round_f32r(
        np.asarray(Wo).reshape(DIM, 8, 256).transpose(1, 0, 2).reshape(8 * DIM, 256))

    u = np.arange(128)[:, None]
    maskB = (np.arange(128)[None, :] < u).astype(np.float32)        # qq>=u -> 0
    maskA = (u <= np.arange(128)[None, :]).astype(np.float32)       # u>qq -> 0

    rotp = np.zeros((128, 128), np.float32)
    d = np.arange(128)
    rotp[(d + 64) % 128, d] = 1.0  # out[d] = in[(d+64)%128]

    ones = np.ones((128, 128), np.float32)

    in_maps = []
    for c in range(N_CORES):
        qs = c * TQ
        xq = x[qs:qs + TQ]                      # [512, 2048]
        xkv = np.zeros((TKV, DIM), np.float32)  # [1536, 2048]
        lo = qs - WIN
        src_lo = max(0, lo)
        xkv[src_lo - lo:TKV] = x[src_lo:qs + TQ]

        pos_q = np.arange(qs, qs + TQ, dtype=np.float64)
        pos_k = np.arange(lo, qs + TQ, dtype=np.float64)
        angq = dfreq[:, None] * pos_q[None, :]  # [128, 512]
        angk = dfreq[:, None] * pos_k[None, :]  # [128, 1536]
        sgn = np.where(np.arange(D) < D // 2, -1.0, 1.0)[:, None]

        kb = np.zeros((128, NMT), np.float32)
        for m in range(NMT):
            t_abs = 128 * m + np.arange(128)
            kb[:, m] = np.where(t_abs < WIN - qs, -30.0, 0.0)

        in_maps.append({
            "xqT": round_f32r(xq.T),
            "xkvT": round_f32r(
                xkv.T.reshape(DIM, 3, 512).transpose(1, 0, 2).reshape(3 * DIM, 512)),
            "wq": wq_r, "wk": wk_r, "wv": wv_r, "wo": wo_r,  # wq/wo pre-paired
            "cosq": np.cos(angq).astype(np.float32),
            "sinq": (sgn * np.sin(angq)).astype(np.float32),
            "cosk": np.ascontiguousarray(np.cos(angk).astype(np.float32)
                .reshape(D, 3, 512).transpose(1, 0, 2)).reshape(3 * D, 512),
            "sink": np.ascontiguousarray(((sgn * np.sin(angk)).astype(np.float32))
                .reshape(D, 3, 512).transpose(1, 0, 2)).reshape(3 * D, 512),
            "kbias": kb,
            "maskB": maskB, "maskA": maskA,
            "rotp": round_f32r(rotp),
            "ones": round_f32r(ones),
        })
    return in_maps


def kernel(x, Wq, Wk, Wv, Wo, _trace=False, _trace_kwargs=None):
    nc = _build()
    in_maps = _host_inputs(x, Wq, Wk, Wv, Wo)
    res = run_bass_kernel_spmd(nc, in_maps, core_ids=list(range(N_CORES)),
                               trace=_trace, **(_trace_kwargs or {}))
    out = np.empty((1, T, DIM), np.float32)
    for c in range(N_CORES):
        out[0, c * TQ:(c + 1) * TQ, :] = res.results[c]["outT"].T
    if _trace:
        kernel.last_results = res
    return out



# revision 2
# speedup vs baseline: 1.3434x; 1.3434x over previous
"""Sliding-window GQA attention (T=4096, DIM=2048, H=16, KVH=4, D=128, W=1024)
as an 8-core SPMD Trainium2 Bass/Tile kernel.

Sharding: sequence-parallel. Core c owns queries [512c, 512c+512) and
recomputes K/V for its sliding window (1536 kv slots, zero-padded before
position 0). No collectives.

v2 (bf16): all matmul operands bf16 (FWL weight loads, half DMA), RoPE
rotate done with partition-offset DVE ops instead of a matmul, softmax
denominator via DVE accumulation of P tiles + one ones-matmul per head,
Wo prefetched during attention, DMA layouts packed to >=2KB lines.

Dataflow (everything transposed so softmax needs no cross-partition max):
  Q^T[h] [d=128, q=512]   = RoPE(Wq_h^T x_q^T)        (per head)
  K^T[kvh] [128, 1536]    = RoPE(Wk_kvh^T x_kv^T)
  V[m] [t=128, 512=kvh*d] = per t-tile natural layout
  S^T [t-tile, q-span]    = K-tile(stationary) @ Q^T   (PSUM)
  P^T = exp(scale*S^T + kbias[t])   (ACT, bf16 out; kbias kills padded t)
  P^T *= triangle masks on boundary blocks (DVE)
  Y^T[h] += V-tile @ P^T                               (PSUM accumulate)
  pacc += P^T (DVE);  den[h] = ones @ pacc             (one MM per head)
  Y^T[h] = Y^T * (1/den)                               (softmax normalize)
  O^T[e-pair] += Wo-chunk(stationary) @ Y^T[h]         -> DRAM bf16
"""

import math
import os
import sys

import numpy as np


def _ensure_paths():
    for p in (
        "/root/.axon_site",
        "/root/.axon_site/_ro/trn_rl_repo",
        "/root/.axon_site/_ro/pypackages",
        "/opt/trn_rl_repo",
        "/opt/pypackages",
    ):
        if os.path.isdir(p) and p not in sys.path:
            sys.path.append(p)


try:
    import concourse.bass as bass  # noqa: F401
except ImportError:
    _ensure_paths()

import ml_dtypes
import concourse.bass as bass  # noqa: F401
import concourse.mybir as mybir
import concourse.tile as tile
from concourse import bacc
from concourse.bass_utils import run_bass_kernel_spmd

BF16NP = np.dtype(ml_dtypes.bfloat16)

# ---------------------------------------------------------------- constants
N_CORES = 8
T = 4096
DIM = 2048
H = 16
KVH = 4
D = 128
WIN = 1024
ROPE_BASE = 10000.0

TQ = T // N_CORES          # 512 queries per core
TKV = TQ + WIN             # 1536 kv slots per core
NMT = TKV // 128           # 12 kv tiles of 128
NCC = DIM // 128           # 16 contraction chunks
SCALE = 1.0 / math.sqrt(D)
GQ = H // KVH              # 4 q heads per kv head

F32 = mybir.dt.float32
BF16 = mybir.dt.bfloat16

# per kv-tile m: (qlo, qhi) span of local queries it can interact with
SPANS = {
    0: (0, 128), 1: (0, 256), 2: (0, 384), 3: (0, 512),
    4: (0, 512), 5: (0, 512), 6: (0, 512), 7: (0, 512),
    8: (0, 512), 9: (128, 512), 10: (256, 512), 11: (384, 512),
}
# per kv-tile m: (mask_name, lo, hi) in local q coords, or None
MASKS = {
    0: ("maskB", 0, 128), 1: ("maskB", 128, 256),
    2: ("maskB", 256, 384), 3: ("maskB", 384, 512),
    4: None, 5: None, 6: None, 7: None,
    8: ("maskA", 0, 128), 9: ("maskA", 128, 256),
    10: ("maskA", 256, 384), 11: ("maskA", 384, 512),
}
# PSUM accumulation order: m=4 first (full-width span -> start=True clears
# the whole Y bank), m=11 last (stop=True).
M_ORDER = [4, 5, 6, 7, 0, 1, 2, 3, 8, 9, 10, 11]


# ---------------------------------------------------------------- device code
_NC_CACHE = None


def _build():
    global _NC_CACHE
    if _NC_CACHE is not None:
        return _NC_CACHE

    nc = bacc.Bacc("TRN2", target_bir_lowering=False, debug=False,
                   num_devices=N_CORES)

    # DRAM I/O (per-core contents supplied via in_maps)
    # xT: x for the core's kv window, transposed: [2048 dims, 1536 pos] bf16;
    #     columns [1024:1536] are the core's own queries.
    xT = nc.dram_tensor("xT", [DIM, TKV], BF16, kind="ExternalInput").ap()
    # wkP/wvP: 8 tiles [128, 1024], tile t = [chunk t | chunk t+8]
    wk = nc.dram_tensor("wk", [8 * 128, 1024], BF16, kind="ExternalInput").ap()
    wv = nc.dram_tensor("wv", [8 * 128, 1024], BF16, kind="ExternalInput").ap()
    # wqP: per head-pair p, 4 tiles [128,1024]; tile cg packs chunks 4cg+k
    wq = nc.dram_tensor("wq", [8 * 4 * 128, 1024], BF16,
                        kind="ExternalInput").ap()
    # woP: per e-pair np, 4 tiles [128,1024]; tile hg packs h-chunks 4hg+k
    wo = nc.dram_tensor("wo", [8 * 4 * 128, 1024], BF16,
                        kind="ExternalInput").ap()
    cosT = nc.dram_tensor("cosT", [D, TKV], F32, kind="ExternalInput").ap()
    sinT = nc.dram_tensor("sinT", [D, TKV], F32, kind="ExternalInput").ap()
    kbias = nc.dram_tensor("kbias", [128, NMT], F32, kind="ExternalInput").ap()
    maskB = nc.dram_tensor("maskB", [128, 128], BF16, kind="ExternalInput").ap()
    maskA = nc.dram_tensor("maskA", [128, 128], BF16, kind="ExternalInput").ap()
    ones = nc.dram_tensor("ones", [128, 128], BF16, kind="ExternalInput").ap()
    # outP: row block b in [0,8): [128, 1024] = [e-tile 2b | e-tile 2b+1]
    outP = nc.dram_tensor("outP", [8 * 128, 1024], BF16,
                          kind="ExternalOutput").ap()

    mask_dram = {"maskB": maskB, "maskA": maskA}

    with tile.TileContext(nc) as tc:
        _emit(nc, tc, xT, wk, wv, wq, wo, cosT, sinT, kbias, mask_dram,
              ones, outP)

    nc.compile()
    _NC_CACHE = nc
    return nc


def _emit(nc, tc, xT, wk, wv, wq, wo, cosT, sinT, kbias, mask_dram, ones,
          outP):
    from contextlib import ExitStack

    ctx = ExitStack()
    with ctx:
        # ---- persistent pools
        consts = ctx.enter_context(tc.tile_pool(name="consts", bufs=1))
        xt = ctx.enter_context(tc.tile_pool(name="xt", bufs=NCC))
        ktp = ctx.enter_context(tc.tile_pool(name="ktp", bufs=KVH))
        vp = ctx.enter_context(tc.tile_pool(name="vp", bufs=NMT))
        ytp = ctx.enter_context(tc.tile_pool(name="ytp", bufs=H))
        qtp = ctx.enter_context(tc.tile_pool(name="qtp", bufs=4))
        pp = ctx.enter_context(tc.tile_pool(name="pp", bufs=3))
        pap = ctx.enter_context(tc.tile_pool(name="pap", bufs=2))
        tmp = ctx.enter_context(tc.tile_pool(name="tmp", bufs=4))
        fin = ctx.enter_context(tc.tile_pool(name="fin", bufs=3))
        ps_a = ctx.enter_context(tc.tile_pool(name="ps_a", bufs=2, space="PSUM"))
        ps_b = ctx.enter_context(tc.tile_pool(name="ps_b", bufs=2, space="PSUM"))
        ps_s = ctx.enter_context(tc.tile_pool(name="ps_s", bufs=2, space="PSUM"))
        ps_y = ctx.enter_context(tc.tile_pool(name="ps_y", bufs=2, space="PSUM"))

        Exp = mybir.ActivationFunctionType.Exp

        # ---- constants into SBUF
        def cload(ap, shape, dtype, tag):
            t = consts.tile(shape, dtype, tag=tag)
            nc.sync.dma_start(t[:], ap[:])
            return t

        ones_sb = cload(ones, [128, 128], BF16, "ones")
        kbias_sb = cload(kbias, [128, NMT], F32, "kbias")
        cos_sb = cload(cosT, [D, TKV], F32, "cosT")
        sin_sb = cload(sinT, [D, TKV], F32, "sinT")
        mask_sb = {
            name: cload(mask_dram[name], [128, 128], BF16, name)
            for name in ("maskB", "maskA")
        }

        def rope(src_ps, lo, width, dst_ap):
            """dst = src*cos + rot_half(src)*sin  (dst bf16).

            rot_half via partition-offset DVE reads: out[d] = src[(d+64)%128],
            sin is sign-folded on host. cos/sin columns [lo, lo+width).
            """
            t1 = tmp.tile([128, 512], F32, tag="t1")
            nc.vector.tensor_mul(t1[0:64, :width], src_ps[64:128, :width],
                                 sin_sb[0:64, lo:lo + width])
            nc.vector.tensor_mul(t1[64:128, :width], src_ps[0:64, :width],
                                 sin_sb[64:128, lo:lo + width])
            t2 = tmp.tile([128, 512], F32, tag="t2")
            nc.vector.tensor_mul(t2[:, :width], src_ps[:, :width],
                                 cos_sb[:, lo:lo + width])
            nc.vector.tensor_add(dst_ap, t1[:, :width], t2[:, :width])

        # ---- persistent K^T / V / Y^T tiles
        kt_sb = [ktp.tile([128, TKV], BF16, tag="kt", name=f"kt{g}")
                 for g in range(KVH)]
        v_sb = [vp.tile([128, 512], BF16, tag="v", name=f"v{m}")
                for m in range(NMT)]
        yt_sb = [ytp.tile([128, TQ], BF16, tag="yt", name=f"yt{h}")
                 for h in range(H)]

        # ---- phase A: K^T (RoPE'd) and V over 3 spans of 512 kv slots
        with tc.tile_pool(name="wkv", bufs=16) as wkv:
            wk_sb = []
            for t in range(8):
                wt = wkv.tile([128, 1024], BF16, tag="wkv", name=f"wk{t}")
                nc.sync.dma_start(wt[:], wk[t * 128:(t + 1) * 128, :])
                wk_sb.append(wt)
            xt_sb = []
            for c in range(NCC):
                x = xt.tile([128, TKV], BF16, tag="xt", name=f"xt{c}")
                nc.sync.dma_start(x[:], xT[c * 128:(c + 1) * 128, :])
                xt_sb.append(x)
            wv_sb = []
            for t in range(8):
                wt = wkv.tile([128, 1024], BF16, tag="wkv", name=f"wv{t}")
                nc.sync.dma_start(wt[:], wv[t * 128:(t + 1) * 128, :])
                wv_sb.append(wt)

            def wk_sl(c, g):
                return wk_sb[c % 8][:, (c // 8) * 512 + g * 128:
                                    (c // 8) * 512 + (g + 1) * 128]

            def wv_sl(c):
                return wv_sb[c % 8][:, (c // 8) * 512:(c // 8) * 512 + 512]

            for s in range(3):
                lo = s * 512
                # K^T projection: c-outer across 4 psum banks
                kps = [ps_s.tile([128, 512], F32, tag="ps_s", name=f"kps{s}_0"),
                       ps_s.tile([128, 512], F32, tag="ps_s", name=f"kps{s}_1"),
                       ps_y.tile([128, 512], F32, tag="ps_y", name=f"kps{s}_2"),
                       ps_y.tile([128, 512], F32, tag="ps_y", name=f"kps{s}_3")]
                for c in range(NCC):
                    for g in range(KVH):
                        nc.tensor.matmul(kps[g][:], wk_sl(c, g),
                                         xt_sb[c][:, lo:lo + 512],
                                         start=(c == 0), stop=(c == NCC - 1))
                for g in range(KVH):
                    rope(kps[g], lo, 512, kt_sb[g][:, lo:lo + 512])

                # V projection (natural layout): c-outer across 4 psum banks
                vps = [ps_a.tile([128, 512], F32, tag="ps_a", name=f"vps{s}_0"),
                       ps_a.tile([128, 512], F32, tag="ps_a", name=f"vps{s}_1"),
                       ps_b.tile([128, 512], F32, tag="ps_b", name=f"vps{s}_2"),
                       ps_b.tile([128, 512], F32, tag="ps_b", name=f"vps{s}_3")]
                for c in range(NCC):
                    for tt in range(4):
                        nc.tensor.matmul(
                            vps[tt][:],
                            xt_sb[c][:, lo + tt * 128:lo + (tt + 1) * 128],
                            wv_sl(c),
                            start=(c == 0), stop=(c == NCC - 1))
                for tt in range(4):
                    nc.vector.tensor_copy(v_sb[4 * s + tt][:], vps[tt][:])

        # ---- phases B+C interleaved per head, with Wo prefetch
        with tc.tile_pool(name="wqp", bufs=8) as wqp, \
                tc.tile_pool(name="wop", bufs=12) as wop:
            qts = {}
            wo_sb = {}
            wo_issued = [0]

            def issue_wo(n):
                """Prefetch the next n wo tiles (4 per e-pair, 32 total)."""
                for _ in range(n):
                    i = wo_issued[0]
                    if i >= 32:
                        return
                    wo_issued[0] += 1
                    wt = wop.tile([128, 1024], BF16, tag="wo", name=f"wo{i}")
                    nc.gpsimd.dma_start(wt[:], wo[i * 128:(i + 1) * 128, :])
                    wo_sb[i] = wt

            def emit_pair_proj(p_):
                h0 = 2 * p_
                wq_t = []
                for cg in range(4):
                    wt = wqp.tile([128, 1024], BF16, tag="wq",
                                  name=f"wqt{p_}_{cg}")
                    nc.sync.dma_start(
                        wt[:],
                        wq[(p_ * 4 + cg) * 128:(p_ * 4 + cg + 1) * 128, :])
                    wq_t.append(wt)
                qpair = [ps_a.tile([128, 512], F32, tag="ps_a",
                                   name=f"qps{h0}_{j}") for j in range(2)]
                for c in range(NCC):
                    cg, k = c // 4, c % 4
                    for j in range(2):
                        nc.tensor.matmul(
                            qpair[j][:],
                            wq_t[cg][:, k * 256 + j * 128:k * 256 + (j + 1) * 128],
                            xt_sb[c][:, WIN:TKV],
                            start=(c == 0), stop=(c == NCC - 1))
                for j in range(2):
                    qtj = qtp.tile([128, TQ], BF16, tag="qt", name=f"qt{h0}_{j}")
                    rope(qpair[j], WIN, TQ, qtj[:])
                    qts[h0 + j] = qtj

            def emit_attn(h):
                g = h // GQ
                qt = qts[h]
                yps = ps_y.tile([128, TQ], F32, tag="ps_y", name=f"yps{h}")
                pacc = pap.tile([128, TQ], BF16, tag="pacc", name=f"pacc{h}")
                for mi, m in enumerate(M_ORDER):
                    qlo, qhi = SPANS[m]
                    w = qhi - qlo
                    sps = ps_s.tile([128, 512], F32, tag="ps_s",
                                    name=f"sps{h}_{m}")
                    nc.tensor.matmul(sps[:, :w],
                                     kt_sb[g][:, m * 128:(m + 1) * 128],
                                     qt[:, qlo:qhi], start=True, stop=True)
                    p = pp.tile([128, 512], BF16, tag="p", name=f"p{h}_{m}")
                    nc.scalar.activation(p[:, :w], sps[:, :w], Exp,
                                         bias=kbias_sb[:, m:m + 1], scale=SCALE)
                    mk = MASKS[m]
                    if mk is not None:
                        name, mlo, mhi = mk
                        nc.vector.tensor_mul(p[:, mlo - qlo:mhi - qlo],
                                             p[:, mlo - qlo:mhi - qlo],
                                             mask_sb[name][:])
                    first = mi == 0
                    last = mi == len(M_ORDER) - 1
                    nc.tensor.matmul(yps[:, qlo:qhi],
                                     v_sb[m][:, g * 128:(g + 1) * 128],
                                     p[:, :w], start=first, stop=last)
                    if first:
                        nc.vector.tensor_copy(pacc[:], p[:])
                    else:
                        nc.vector.tensor_add(pacc[:, qlo:qhi],
                                             pacc[:, qlo:qhi], p[:, :w])

                dps = ps_b.tile([128, TQ], F32, tag="ps_b", name=f"dps{h}")
                nc.tensor.matmul(dps[:], ones_sb[:], pacc[:],
                                 start=True, stop=True)
                rcp = fin.tile([128, TQ], F32, tag="rcp", name=f"rcp{h}")
                nc.vector.reciprocal(rcp[:], dps[:])
                nc.vector.tensor_mul(yt_sb[h][:], yps[:], rcp[:])

            # one-pair lookahead: emit projections a pair ahead of attention
            emit_pair_proj(0)
            for p_ in range(H // 2):
                if p_ + 1 < H // 2:
                    emit_pair_proj(p_ + 1)
                issue_wo(2)
                emit_attn(2 * p_)
                issue_wo(2)
                emit_attn(2 * p_ + 1)

            # ---- phase D: O^T projection in e-tile pairs
            for np_ in range(8):
                opair = [ps_a.tile([128, 512], F32, tag="ps_a",
                                   name=f"ops{np_}_{j}") for j in range(2)]
                for hg in range(4):
                    wot = wo_sb[np_ * 4 + hg]
                    for k in range(4):
                        h = 4 * hg + k
                        for j in range(2):
                            nc.tensor.matmul(
                                opair[j][:],
                                wot[:, k * 256 + j * 128:k * 256 + (j + 1) * 128],
                                yt_sb[h][:],
                                start=(h == 0), stop=(h == H - 1))
                osb = fin.tile([128, 1024], BF16, tag="osb", name=f"osb{np_}")
                nc.vector.tensor_copy(osb[:, 0:512], opair[0][:])
                nc.vector.tensor_copy(osb[:, 512:1024], opair[1][:])
                nc.sync.dma_start(outP[np_ * 128:(np_ + 1) * 128, :], osb[:])


# ---------------------------------------------------------------- host side
def _host_inputs(x, Wq, Wk, Wv, Wo):
    x = np.asarray(x, dtype=np.float32).reshape(T, DIM)
    Wq = np.asarray(Wq, dtype=np.float32)
    Wk = np.asarray(Wk, dtype=np.float32)
    Wv = np.asarray(Wv, dtype=np.float32)
    Wo = np.asarray(Wo, dtype=np.float32)

    inv_freq = 1.0 / (ROPE_BASE ** (np.arange(0, D, 2, dtype=np.float64) / D))
    dfreq = np.concatenate([inv_freq, inv_freq])  # [128] per-dim freq

    # wk/wv: 8 tiles [128, 1024] = [chunk t | chunk t+8]
    def pack_kv(W):
        ch = W.reshape(NCC, 128, KVH * D)          # [16, 128, 512]
        out = np.empty((8 * 128, 1024), np.float32)
        for t in range(8):
            out[t * 128:(t + 1) * 128, 0:512] = ch[t]
            out[t * 128:(t + 1) * 128, 512:1024] = ch[t + 8]
        return out.astype(BF16NP)

    wk_p = pack_kv(Wk)
    wv_p = pack_kv(Wv)

    # wq: per pair p, 4 tiles [128,1024]; tile cg = concat_k chunk(4cg+k)
    # of Wq[:, p*256:(p+1)*256]
    wq_p = np.empty((8 * 4 * 128, 1024), np.float32)
    for p in range(8):
        wp = Wq[:, p * 256:(p + 1) * 256]          # [2048, 256]
        ch = wp.reshape(NCC, 128, 256)
        for cg in range(4):
            blk = np.concatenate([ch[4 * cg + k] for k in range(4)], axis=1)
            wq_p[(p * 4 + cg) * 128:(p * 4 + cg + 1) * 128, :] = blk
    wq_p = wq_p.astype(BF16NP)

    # wo: per e-pair np, 4 tiles [128,1024]; tile hg = concat_k h-chunk(4hg+k)
    # of Wo[:, np*256:(np+1)*256]
    wo_p = np.empty((8 * 4 * 128, 1024), np.float32)
    for np_ in range(8):
        wp = Wo[:, np_ * 256:(np_ + 1) * 256]      # [2048, 256]
        ch = wp.reshape(H, 128, 256)
        for hg in range(4):
            blk = np.concatenate([ch[4 * hg + k] for k in range(4)], axis=1)
            wo_p[(np_ * 4 + hg) * 128:(np_ * 4 + hg + 1) * 128, :] = blk
    wo_p = wo_p.astype(BF16NP)

    u = np.arange(128)[:, None]
    maskB = (np.arange(128)[None, :] < u).astype(np.float32)   # q < t keeps
    maskA = (u <= np.arange(128)[None, :]).astype(np.float32)  # q >= t keeps
    ones = np.ones((128, 128), np.float32)

    in_maps = []
    for c in range(N_CORES):
        qs = c * TQ
        xkv = np.zeros((TKV, DIM), np.float32)  # [1536, 2048]
        lo = qs - WIN
        src_lo = max(0, lo)
        xkv[src_lo - lo:TKV] = x[src_lo:qs + TQ]

        pos_k = np.arange(lo, qs + TQ, dtype=np.float64)
        angk = dfreq[:, None] * pos_k[None, :]  # [128, 1536]
        sgn = np.where(np.arange(D) < D // 2, -1.0, 1.0)[:, None]

        kb = np.zeros((128, NMT), np.float32)
        for m in range(NMT):
            t_abs = 128 * m + np.arange(128)
            kb[:, m] = np.where(t_abs < WIN - qs, -30.0, 0.0)

        in_maps.append({
            "xT": np.ascontiguousarray(xkv.T).astype(BF16NP),
            "wk": wk_p, "wv": wv_p, "wq": wq_p, "wo": wo_p,
            "cosT": np.cos(angk).astype(np.float32),
            "sinT": (sgn * np.sin(angk)).astype(np.float32),
            "kbias": kb,
            "maskB": maskB.astype(BF16NP), "maskA": maskA.astype(BF16NP),
            "ones": ones.astype(BF16NP),
        })
    return in_maps


def kernel(x, Wq, Wk, Wv, Wo, _trace=False, _trace_kwargs=None):
    nc = _build()
    in_maps = _host_inputs(x, Wq, Wk, Wv, Wo)
    res = run_bass_kernel_spmd(nc, in_maps, core_ids=list(range(N_CORES)),
                               trace=_trace, **(_trace_kwargs or {}))
    out = np.empty((1, T, DIM), np.float32)
    for c in range(N_CORES):
        op = np.asarray(res.results[c]["outP"], dtype=np.float32)
        # outP row block b: [128, 1024] = [e-tile 2b (cols 0:512) | 2b+1]
        op = op.reshape(8, 128, 2, 512)            # [b, p, j, q]
        oT = op.transpose(0, 2, 1, 3).reshape(DIM, TQ)  # [e, q]
        out[0, c * TQ:(c + 1) * TQ, :] = oT.T
    if _trace:
        kernel.last_results = res
    return out
